# revision 1
# baseline (speedup 1.0000x reference)
"""DRM attention kernel for 8 Trainium2 NeuronCores.

Sharding: B*H = 32 head-slices; core c handles batch b = c//4 and the 4
heads [4*(c%4), 4*(c%4)+4). Weights replicated (pre-sliced per core on
host). Each core computes its 4 heads' attention output through Wo,
producing a partial [T, DM] for its batch; host sums the 4 partials per
batch (the unshard/reduce step).

All on-device score work happens in transposed layout S^T[j, i] (j = key
pos on partitions, i = query pos on free dim) so the probability tiles
feed the attention*V matmul directly as the stationary operand.

dist(i,j) = |qm_i|^2 + |km_j|^2 - 2 qm.km                  (euclidean)
          + |Uq_i|^2 - 2 sum_r Uq_ir Uk_ijr + sum_r Uk_ijr^2   (low rank)
with Uk_ijr = sum_d U[i,d,r] km[j,d].  The cross term folds into the
linear matmul via w'[i,d] = sum_r U[i,d,r] Uq[i,r]:
  S_lin[j,i] = (-2 km_j).(qm_i + w'_i) + 1*(|qm_i|^2+|Uq_i|^2) + |km_j|^2*1
realized as one K=34 matmul; the quadratic term adds 4 K=32 matmuls
(Uk_r) squared and accumulated per block.  Softmax skips the max
subtraction (all logits <= 0) and gets its denominator from a ones
column appended to V.
"""

import numpy as np

B, T, DM = 2, 512, 1024
H, DH = 16, 64
D, R = 32, 4
TEMP_MIN = 0.5
NCORE = 8
HPC = 4          # heads per core
TC = 4           # 128-chunks along T

_CACHE = {}


def _build():
    import concourse.bass as bass
    import concourse.tile as tile
    from concourse import mybir, bacc

    f32 = mybir.dt.float32
    PSUM = bass.MemorySpace.PSUM
    Alu = mybir.AluOpType
    Act = mybir.ActivationFunctionType
    AxX = mybir.AxisListType.X

    nc = bacc.Bacc("TRN2", target_bir_lowering=False, debug=False)
    f32r = mybir.dt.float32r

    def mm(out, lhsT, rhs, **kw):
        # float32r: same fp32 bits, single-pass PE (4x faster than fp32)
        nc.tensor.matmul(out, lhsT.bitcast(f32r), rhs.bitcast(f32r), **kw)

    xt_d = nc.dram_tensor("xt", [DM, T], f32r, kind="ExternalInput")
    wqk_d = nc.dram_tensor("wqk", [DM, 512], f32r, kind="ExternalInput")
    wv_d = nc.dram_tensor("wv", [DM, 256], f32r, kind="ExternalInput")
    wo_d = nc.dram_tensor("wo", [256, DM], f32r, kind="ExternalInput")
    bqkA_d = nc.dram_tensor("bqkA", [128, 128], f32r, kind="ExternalInput")
    bqkB_d = nc.dram_tensor("bqkB", [128, 128], f32r, kind="ExternalInput")
    wm_d = nc.dram_tensor("wm", [D, 128], f32r, kind="ExternalInput")
    cosr_d = nc.dram_tensor("cosr", [128, T], f32, kind="ExternalInput")
    sinr_d = nc.dram_tensor("sinr", [128, T], f32, kind="ExternalInput")
    maskd_d = nc.dram_tensor("maskd", [128, 128], f32, kind="ExternalInput")
    nit_d = nc.dram_tensor("nit", [128, 1], f32, kind="ExternalInput")
    i4rep_d = nc.dram_tensor("i4rep", [D, 128], f32r, kind="ExternalInput")
    onesrow_d = nc.dram_tensor("onesrow", [1, T], f32r, kind="ExternalInput")
    gsum_d = nc.dram_tensor("gsum", [128, 128], f32r, kind="ExternalInput")
    bsum_d = nc.dram_tensor("bsum", [128, D], f32r, kind="ExternalInput")
    wb1_d = nc.dram_tensor("wb1", [128, 2], f32r, kind="ExternalInput")
    wb2_d = nc.dram_tensor("wb2", [D, 2], f32r, kind="ExternalInput")
    b01_d = nc.dram_tensor("b01", [2, 1], f32, kind="ExternalInput")
    y_d = nc.dram_tensor("y", [T, DM], f32, kind="ExternalOutput")

    with tile.TileContext(nc) as tc:
        with (
            tc.tile_pool(name="const", bufs=1) as cpool,
            tc.tile_pool(name="rope", bufs=4) as rpool,
            tc.tile_pool(name="qkm", bufs=2) as qkmpool,
            tc.tile_pool(name="uu", bufs=2) as uupool,
            tc.tile_pool(name="ext", bufs=2) as extpool,
            tc.tile_pool(name="scr", bufs=6) as scr,
            tc.tile_pool(name="pt", bufs=3) as ptpool,
            tc.tile_pool(name="stk", bufs=2) as stkpool,
            tc.tile_pool(name="psA", bufs=2, space=PSUM) as psA,
            tc.tile_pool(name="psT", bufs=1, space=PSUM) as psT,
            tc.tile_pool(name="psS", bufs=2, space=PSUM) as psS,
            tc.tile_pool(name="psU", bufs=2, space=PSUM) as psU,
            tc.tile_pool(name="psO", bufs=1, space=PSUM) as psO,
        ):
            # ---- constants / weights ----
            xt = [cpool.tile([128, T], f32r, tag=f"xt{k}", name=f"xt{k}") for k in range(8)]
            wqk = [cpool.tile([128, 512], f32r, tag=f"wqk{k}", name=f"wqk{k}") for k in range(8)]
            wv = [cpool.tile([128, 256], f32r, tag=f"wv{k}", name=f"wv{k}") for k in range(8)]
            wo = [cpool.tile([128, DM], f32r, tag=f"wo{p}", name=f"wo{p}") for p in range(2)]
            bqkA = cpool.tile([128, 128], f32r, tag="bqkA")
            bqkB = cpool.tile([128, 128], f32r, tag="bqkB")
            wm = cpool.tile([D, 128], f32r, tag="wm")
            cosr = cpool.tile([128, T], f32, tag="cosr")
            sinr = cpool.tile([128, T], f32, tag="sinr")
            maskd = cpool.tile([128, 128], f32, tag="maskd")
            nit = cpool.tile([128, 1], f32, tag="nit")
            i4rep = cpool.tile([D, 128], f32r, tag="i4rep")
            gsum = cpool.tile([128, 128], f32r, tag="gsum")
            bsum = cpool.tile([128, D], f32r, tag="bsum")
            wb1 = cpool.tile([128, 2], f32r, tag="wb1")
            wb2 = cpool.tile([D, 2], f32r, tag="wb2")
            b01 = cpool.tile([2, 1], f32, tag="b01")
            ones64 = cpool.tile([1, 64], f32r, tag="ones64")
            ones32 = cpool.tile([D, 1], f32r, tag="ones32")
            vext = cpool.tile([128, TC, 260], f32r, tag="vext")

            xt_r = xt_d.ap().rearrange("(k p) t -> k p t", p=128)
            wqk_r = wqk_d.ap().rearrange("(k p) m -> k p m", p=128)
            wv_r = wv_d.ap().rearrange("(k p) m -> k p m", p=128)
            wo_r = wo_d.ap().rearrange("(k p) m -> k p m", p=128)
            for k in range(8):
                nc.sync.dma_start(xt[k][:], xt_r[k])
                nc.sync.dma_start(wqk[k][:], wqk_r[k])
            nc.sync.dma_start(cosr[:], cosr_d.ap())
            nc.sync.dma_start(sinr[:], sinr_d.ap())
            nc.sync.dma_start(bqkA[:], bqkA_d.ap())
            nc.sync.dma_start(bqkB[:], bqkB_d.ap())
            nc.sync.dma_start(wm[:], wm_d.ap())
            nc.sync.dma_start(i4rep[:], i4rep_d.ap())
            nc.sync.dma_start(gsum[:], gsum_d.ap())
            nc.sync.dma_start(bsum[:], bsum_d.ap())
            nc.sync.dma_start(wb1[:], wb1_d.ap())
            nc.sync.dma_start(wb2[:], wb2_d.ap())
            nc.sync.dma_start(b01[:], b01_d.ap())
            nc.sync.dma_start(maskd[:], maskd_d.ap())
            nc.sync.dma_start(nit[:], nit_d.ap())
            for k in range(8):
                nc.sync.dma_start(wv[k][:], wv_r[k])
            for p in range(2):
                nc.sync.dma_start(wo[p][:], wo_r[p])
            nc.gpsimd.memset(ones64[:].bitcast(f32), 1.0)
            nc.gpsimd.memset(ones32[:].bitcast(f32), 1.0)
            nc.gpsimd.memset(vext[:].bitcast(f32), 1.0)

            # ---- QK projection (4 M-chunks) + RoPE + qm/km, per pair ----
            qkm_sig = []     # per pair: [128,T] = [qm_e0; qm_e1; km_e0; km_e1]
            for p in range(2):
                ropeAB = []
                for s in range(2):      # half: A (first 32 dh) / B (second)
                    m = 2 * p + s
                    qk_ps = psA.tile([128, T], f32, tag="psA")
                    for k in range(8):
                        mm(
                            qk_ps[:], wqk[k][:, m * 128:(m + 1) * 128], xt[k][:],
                            start=(k == 0), stop=(k == 7))
                    ropeAB.append(qk_ps)
                A, Bt = ropeAB
                m1 = scr.tile([128, T], f32, tag="scr", bufs=4)
                m2 = scr.tile([128, T], f32, tag="scr", bufs=4)
                nc.vector.tensor_mul(m1[:], A[:], cosr[:])
                nc.vector.tensor_mul(m2[:], Bt[:], sinr[:])
                ropeA = rpool.tile([128, T], f32r, tag="rope")
                nc.vector.tensor_sub(ropeA[:], m1[:], m2[:])
                m3 = scr.tile([128, T], f32, tag="scr", bufs=4)
                m4 = scr.tile([128, T], f32, tag="scr", bufs=4)
                nc.vector.tensor_mul(m3[:], A[:], sinr[:])
                nc.vector.tensor_mul(m4[:], Bt[:], cosr[:])
                ropeB = rpool.tile([128, T], f32r, tag="rope")
                nc.vector.tensor_add(ropeB[:], m3[:], m4[:])

                qkm_ps = psA.tile([128, T], f32, tag="psA")
                mm(qkm_ps[:], bqkA[:], ropeA[:],
                                 start=True, stop=False)
                mm(qkm_ps[:], bqkB[:], ropeB[:],
                                 start=False, stop=True)
                sig = qkmpool.tile([128, T], f32r, tag="qkm")
                nc.scalar.activation(sig[:], qkm_ps[:], Act.Sigmoid)
                qkm_sig.append(sig)

            # ---- V projection into [v_h | 1] blocks of vext ----
            for jc in range(TC):
                v_ps = psA.tile([128, 256], f32, tag="psA")
                for k in range(8):
                    mm(
                        v_ps[:], xt[k][:, jc * 128:(jc + 1) * 128], wv[k][:],
                        start=(k == 0), stop=(k == 7))
                for hl in range(HPC):
                    nc.vector.tensor_copy(vext[:, jc, hl * 65:hl * 65 + 64],
                                           v_ps[:, hl * 64:(hl + 1) * 64])

            # ---- per head ----
            stacked = []
            for p in range(2):
                stk = stkpool.tile([128, T], f32r, tag="stk", name=f"stk{p}")
                stacked.append(stk)

            for hl in range(HPC):
                p, e = hl // 2, hl % 2
                sig = qkm_sig[p]
                # base-partition-0 copies (matmul needs lhsT/rhs aligned)
                qmT_t = scr.tile([D, T], f32r, tag="qmT", bufs=2)
                kmT_t = scr.tile([D, T], f32r, tag="kmT", bufs=2)
                nc.vector.tensor_copy(qmT_t[:], sig[32 * e:32 * e + 32, :])
                nc.vector.tensor_copy(kmT_t[:], sig[64 + 32 * e:96 + 32 * e, :])
                qmT = qmT_t[:]
                kmT = kmT_t[:]

                # U in r-major transposed layout: UUT[(r,d), i], split in
                # two [64,T] tiles so Uk matmul operands align at base 0/32
                uu_sb = []
                for w in range(2):
                    uut_ps = psA.tile([64, T], f32, tag="psA")
                    mm(uut_ps[:], wm[:, w * 64:(w + 1) * 64],
                       qmT, start=True, stop=True)
                    u = uupool.tile([64, T], f32r, tag="uu", name=f"uu{w}", bufs=4)
                    nc.scalar.copy(u[:], uut_ps[:])
                    uu_sb.append(u)

                # km replicated into both 32-partition groups (Uk lhsT)
                kmrep_ps = psA.tile([64, T], f32, tag="psA")
                mm(kmrep_ps[:], i4rep[:, :64], kmT,
                   start=True, stop=True)
                kmrep = scr.tile([64, T], f32r, tag="kmrep", bufs=2)
                nc.scalar.copy(kmrep[:], kmrep_ps[:])

                # Uq / w' / bias entirely in transposed space:
                #   qmrep[(r,d), i] = qmT[d, i]        (i2rep-style matmul)
                #   tmp = UU . qmrep ; UqT_rep = gsum @ tmp   (sum d in block)
                #   tmp2 = UU . UqT_rep ; w'T = bsum @ tmp2   (sum r per d)
                #   biasT = wb1 @ (UqT_rep^2)/1 + wb2 @ qmT^2
                qmrep_ps = psT.tile([128, T], f32, tag="psT")
                mm(qmrep_ps[:], i4rep[:], qmT, start=True, stop=True)
                tmpc = scr.tile([128, T], f32r, tag="tmpc", bufs=2)
                for w in range(2):
                    nc.vector.tensor_mul(tmpc[64 * w:64 * w + 64, :],
                                         uu_sb[w][:],
                                         qmrep_ps[64 * w:64 * w + 64, :])
                uqrep_ps = psT.tile([128, T], f32, tag="psT")
                mm(uqrep_ps[:], gsum[:], tmpc[:], start=True, stop=True)
                tmp2c = scr.tile([128, T], f32r, tag="tmp2c", bufs=2)
                for w in range(2):
                    nc.vector.tensor_mul(tmp2c[64 * w:64 * w + 64, :],
                                         uu_sb[w][:],
                                         uqrep_ps[64 * w:64 * w + 64, :])
                sq2 = scr.tile([128, T], f32r, tag="sq2", bufs=2)
                nc.scalar.square(sq2[:], uqrep_ps[:])
                qmsq = scr.tile([D, T], f32r, tag="qmsq", bufs=2)
                nc.scalar.square(qmsq[:], qmT)

                wpt_ps = psT.tile([D, T], f32, tag="psT")
                mm(wpt_ps[:], bsum[:], tmp2c[:], start=True, stop=True)
                bias_ps = psS.tile([2, T], f32, tag="psS")
                mm(bias_ps[:], wb1[:], sq2[:], start=True, stop=False)
                mm(bias_ps[:], wb2[:], qmsq[:], start=False, stop=True)

                gt = extpool.tile([34, T], f32r, tag="gt")
                nc.vector.tensor_add(gt[:32, :], wpt_ps[:], qmT)
                nc.scalar.activation(gt[32:34, :], bias_ps[:], Act.Identity,
                                     bias=b01[:], scale=1.0)

                # lhsT of the linear matmul: kmT_ext [34, T]
                # rows: 0:32 = -2*km, 32 = |km|^2 (pairs gt's ones row),
                # 33 = ones (pairs gt's bias row)
                kme = extpool.tile([34, T], f32r, tag="kme")
                nc.vector.tensor_scalar_mul(kme[:32, :], kmT, -2.0)
                kmsq = scr.tile([D, T], f32r, tag="kmsq", bufs=2)
                nc.scalar.square(kmsq[:], kmT)
                k2m_ps = psS.tile([1, T], f32, tag="psS")
                mm(k2m_ps[:], ones32[:], kmsq[:],
                                 start=True, stop=True)
                nc.scalar.copy(kme[32:33, :], k2m_ps[:])
                nc.sync.dma_start(kme[33:34, :], onesrow_d.ap())

                # score blocks, exp, attn*v
                ot_ps = psO.tile([128, T], f32, tag="psO")
                for jc in range(TC):
                    ioff = 128 * jc
                    ni = T - ioff
                    s_ps = psS.tile([128, T], f32, tag="psS")
                    mm(
                        s_ps[:, :ni],
                        kme[:, jc * 128:(jc + 1) * 128],
                        gt[:, ioff:],
                        start=True, stop=True)
                    tsq = []
                    uk_pools = [psU, psU, psT, psA]
                    for r in range(R):
                        uk_ps = uk_pools[r].tile([128, T], f32,
                                                 tag=uk_pools[r].name,
                                                 name=f"uk{r}")
                        w, rr = r // 2, r % 2
                        mm(
                            uk_ps[:, :ni],
                            kmrep[rr * 32:(rr + 1) * 32,
                                  jc * 128:(jc + 1) * 128],
                            uu_sb[w][rr * 32:(rr + 1) * 32, ioff:],
                            start=True, stop=True)
                        t = scr.tile([128, T], f32, tag="tsq", bufs=6)
                        nc.scalar.square(t[:, :ni], uk_ps[:, :ni])
                        tsq.append(t)
                    a01 = scr.tile([128, T], f32, tag="acc", bufs=4)
                    a23 = scr.tile([128, T], f32, tag="acc", bufs=4)
                    nc.gpsimd.tensor_add(a01[:, :ni], tsq[0][:, :ni], tsq[1][:, :ni])
                    nc.gpsimd.tensor_add(a23[:, :ni], tsq[2][:, :ni], tsq[3][:, :ni])
                    a03 = scr.tile([128, T], f32, tag="acc", bufs=4)
                    nc.vector.tensor_add(a03[:, :ni], a01[:, :ni], a23[:, :ni])
                    ssb = scr.tile([128, T], f32, tag="ssb", bufs=3)
                    nc.vector.tensor_add(ssb[:, :ni], a03[:, :ni], s_ps[:, :ni])

                    pt = ptpool.tile([128, T], f32r, tag="pt")
                    nc.scalar.activation(pt[:, :ni], ssb[:, :ni], Act.Exp,
                                         scale=nit[:, 0:1])
                    nc.vector.tensor_mul(pt[:, :128], pt[:, :128], maskd[:])
                    mm(
                        ot_ps[:65, ioff:],
                        vext[:, jc, hl * 65:(hl + 1) * 65],
                        pt[:, :ni],
                        start=(jc == 0), stop=(jc == 3),
                        skip_group_check=True)

                # normalize and stack into [o_e0; o_e1] per pair: broadcast
                # the raw denominator row across 64 partitions via a K=1
                # matmul (operand bases must match: ones64[64:65] aligns with
                # the denominator at PSUM partition 64), then reciprocal at
                # base 0 (reciprocal_approx misreads nonzero-base PSUM).
                den_sb = scr.tile([1, T], f32r, tag="den", bufs=2)
                nc.scalar.copy(den_sb[:], ot_ps[64:65, :])
                bc_ps = psS.tile([64, T], f32, tag="psS")
                mm(bc_ps[:], ones64[:], den_sb[:], start=True, stop=True)
                bc_sb = scr.tile([64, T], f32, tag="bcsb", bufs=2)
                nc.vector.reciprocal_approx_fast(out=bc_sb[:], in_=bc_ps[:])
                nc.vector.tensor_mul(stacked[p][64 * e:64 * e + 64, :],
                                     ot_ps[:64, :], bc_sb[:])

            # ---- output projection (partial y for this core's 4 heads) ----
            for ic in range(TC):
                for ncn in range(2):
                    y_ps = psS.tile([128, 512], f32, tag="psS", name="y_ps")
                    mm(
                        y_ps[:], stacked[0][:, ic * 128:(ic + 1) * 128],
                        wo[0][:, ncn * 512:(ncn + 1) * 512],
                        start=True, stop=False)
                    mm(
                        y_ps[:], stacked[1][:, ic * 128:(ic + 1) * 128],
                        wo[1][:, ncn * 512:(ncn + 1) * 512],
                        start=False, stop=True)
                    y_sb = scr.tile([128, 512], f32, tag="ysb", bufs=2)
                    nc.scalar.copy(y_sb[:], y_ps[:])
                    nc.sync.dma_start(
                        y_d.ap()[ic * 128:(ic + 1) * 128,
                                 ncn * 512:(ncn + 1) * 512],
                        y_sb[:])

    nc.compile()
    return nc


def _r32(a):
    """Round fp32 to fp32r (11-bit mantissa, RNE) so on-device fp32r matmuls
    see pre-rounded operands."""
    u = np.ascontiguousarray(a, np.float32).view(np.uint32).astype(np.uint64)
    u = (u + 0x7FF + ((u >> 12) & 1)) & 0xFFFFF000
    return u.astype(np.uint32).view(np.float32)


def _rope_tables():
    inv_freq = 1.0 / (10000.0 ** (np.arange(0, DH, 2, dtype=np.float32) / DH))
    t = np.arange(T, dtype=np.float32)
    freqs = t[:, None] * inv_freq[None, :]          # [T, 32]
    return np.cos(freqs), np.sin(freqs)


def _prep_inputs(x, Wq, Wk, Wv, Wo, Wqm, Wkm, Wmetric, temperature):
    x = np.asarray(x, np.float32)
    Wq, Wk, Wv, Wo = (np.asarray(w, np.float32) for w in (Wq, Wk, Wv, Wo))
    Wqm, Wkm = np.asarray(Wqm, np.float32), np.asarray(Wkm, np.float32)
    Wmetric = np.asarray(Wmetric, np.float32)
    temp = float(np.asarray(temperature))

    cosf, sinf = _rope_tables()
    cosr = np.ascontiguousarray(np.tile(cosf.T, (4, 1)))   # [128, T]
    sinr = np.ascontiguousarray(np.tile(sinf.T, (4, 1)))

    bqkA = np.zeros((128, 128), np.float32)
    bqkB = np.zeros((128, 128), np.float32)
    for ee in range(2):
        bqkA[64 * ee:64 * ee + 32, 32 * ee:32 * ee + 32] = Wqm[0:32]
        bqkA[64 * ee + 32:64 * ee + 64, 64 + 32 * ee:96 + 32 * ee] = Wkm[0:32]
        bqkB[64 * ee:64 * ee + 32, 32 * ee:32 * ee + 32] = Wqm[32:64]
        bqkB[64 * ee + 32:64 * ee + 64, 64 + 32 * ee:96 + 32 * ee] = Wkm[32:64]

    wm = np.ascontiguousarray(
        Wmetric.reshape(D, D, R).transpose(0, 2, 1).reshape(D, D * R))

    jj, ii = np.meshgrid(np.arange(128), np.arange(128), indexing="ij")
    maskd = (jj <= ii).astype(np.float32)
    nit = np.full((128, 1), -1.0 / max(temp, TEMP_MIN), np.float32)
    i4rep = np.ascontiguousarray(np.tile(np.eye(D, dtype=np.float32), (1, 4)))
    rr, dd = np.arange(128) // D if False else np.arange(128) // 32, np.arange(128) % 32
    gsum = np.zeros((128, 128), np.float32)   # [(r',d'), (r,d)] = [r'==r]
    for a in range(128):
        for bcol in range(128):
            if a // 32 == bcol // 32:
                gsum[a, bcol] = 1.0
    bsum = np.zeros((128, D), np.float32)     # [(r,d), d'] = [d==d']
    for a in range(128):
        bsum[a, a % 32] = 1.0
    wb1 = np.zeros((128, 2), np.float32); wb1[:, 1] = 1.0 / 32.0
    wb2 = np.zeros((D, 2), np.float32); wb2[:, 1] = 1.0
    b01 = np.array([[1.0], [0.0]], np.float32)

    in_maps = []
    for c in range(NCORE):
        b, g = c // 4, c % 4
        lh0 = 4 * g
        wqk = np.empty((DM, 512), np.float32)
        for p in range(2):
            for s in range(2):
                m = 2 * p + s
                for ee in range(2):
                    h = lh0 + 2 * p + ee
                    cq = Wq[:, h * 64 + 32 * s: h * 64 + 32 * s + 32]
                    ck = Wk[:, h * 64 + 32 * s: h * 64 + 32 * s + 32]
                    wqk[:, m * 128 + 64 * ee: m * 128 + 64 * ee + 32] = cq
                    wqk[:, m * 128 + 64 * ee + 32: m * 128 + 64 * ee + 64] = ck
        in_maps.append({
            "xt": _r32(x[b].T),
            "wqk": _r32(wqk),
            "wv": _r32(Wv[:, lh0 * 64: lh0 * 64 + 256]),
            "wo": _r32(Wo[lh0 * 64: lh0 * 64 + 256, :]),
            "bqkA": _r32(bqkA),
            "bqkB": _r32(bqkB),
            "wm": _r32(wm),
            "cosr": cosr,
            "sinr": sinr,
            "maskd": maskd,
            "nit": nit,
            "i4rep": i4rep,
            "gsum": gsum, "bsum": bsum, "wb1": wb1, "wb2": wb2, "b01": b01,
            "onesrow": np.ones((1, T), np.float32),
        })
    return in_maps


def kernel(x, Wq, Wk, Wv, Wo, Wqm, Wkm, Wmetric, temperature, **_):
    from concourse import bass_utils

    if "nc" not in _CACHE:
        _CACHE["nc"] = _build()
    nc = _CACHE["nc"]

    in_maps = _prep_inputs(x, Wq, Wk, Wv, Wo, Wqm, Wkm, Wmetric, temperature)
    res = bass_utils.run_bass_kernel_spmd(nc, in_maps,
                                          core_ids=list(range(NCORE)))
    y = np.zeros((B, T, DM), np.float32)
    for b in range(B):
        acc = res.results[4 * b]["y"].astype(np.float32)
        for g in range(1, 4):
            acc = acc + res.results[4 * b + g]["y"]
        y[b] = acc
    return y



# revision 2
# speedup vs baseline: 1.1940x; 1.1940x over previous
"""DRM attention kernel for 8 Trainium2 NeuronCores — v3.

v2 -> v3: pair-tile PSUM layout (2-bank tiles) so ACT squares cover two
uk banks per instruction; all-pairs metric stage hoisted before scores;
software-pipelined score loop (spans -> squares -> tree -> exp -> attnV
one block behind) to keep the PE stream dense for HAM; copies spread
across DVE/ACT/GPS by phase.
"""

import numpy as np
import ml_dtypes

B, T, DM = 2, 512, 1024
H, DH = 16, 64
D, R = 32, 4
TEMP_MIN = 0.5
NCORE = 8
TC = 4

_CACHE = {}
BF16 = ml_dtypes.bfloat16


def _build(temp):
    import concourse.bass as bass
    import concourse.tile as tile
    from concourse import mybir, bacc

    f32 = mybir.dt.float32
    f32r = mybir.dt.float32r
    bf16 = mybir.dt.bfloat16
    PSUM = bass.MemorySpace.PSUM
    Act = mybir.ActivationFunctionType

    it = -1.0 / max(temp, TEMP_MIN)

    nc = bacc.Bacc("TRN2", target_bir_lowering=False, debug=False)

    xt_d = nc.dram_tensor("xt", [DM, T], bf16, kind="ExternalInput")
    wqk_d = nc.dram_tensor("wqk", [DM, 512], bf16, kind="ExternalInput")
    wv_d = nc.dram_tensor("wv", [DM, 256], bf16, kind="ExternalInput")
    wo_d = nc.dram_tensor("wo", [256, DM], bf16, kind="ExternalInput")
    bqkA_d = nc.dram_tensor("bqkA", [128, 128], bf16, kind="ExternalInput")
    bqkAn_d = nc.dram_tensor("bqkAn", [128, 128], bf16, kind="ExternalInput")
    bqkB_d = nc.dram_tensor("bqkB", [128, 128], bf16, kind="ExternalInput")
    cosr_d = nc.dram_tensor("cosr", [128, T], bf16, kind="ExternalInput")
    sinr_d = nc.dram_tensor("sinr", [128, T], bf16, kind="ExternalInput")
    wm4_d = nc.dram_tensor("wm4", [128, 128], f32r, kind="ExternalInput")
    i4q4_d = nc.dram_tensor("i4q4", [128, 128], f32r, kind="ExternalInput")
    i4kn2_d = nc.dram_tensor("i4kn2", [128, 128], f32r, kind="ExternalInput")
    gsum_d = nc.dram_tensor("gsum", [128, 128], f32r, kind="ExternalInput")
    bsum4_d = nc.dram_tensor("bsum4", [128, 128], f32r, kind="ExternalInput")
    maskd_d = nc.dram_tensor("maskd", [128, 128], bf16, kind="ExternalInput")
    y_d = nc.dram_tensor("y", [T, DM], bf16, kind="ExternalOutput")

    with tile.TileContext(nc) as tc:
        with (
            tc.tile_pool(name="const", bufs=1) as cpool,
            tc.tile_pool(name="sig", bufs=1) as sigpool,
            tc.tile_pool(name="m", bufs=8) as mpool,
            tc.tile_pool(name="met", bufs=2) as metpool,
            tc.tile_pool(name="sc", bufs=2) as scpool,
            tc.tile_pool(name="f", bufs=4) as fpool,
            tc.tile_pool(name="pt", bufs=4) as ptpool,
            tc.tile_pool(name="out", bufs=2) as opool,
            tc.tile_pool(name="ps", bufs=1, space=PSUM) as psp,
        ):
            # ---- constants / weights ----
            xt = [cpool.tile([128, T], bf16, tag=f"xt{k}", name=f"xt{k}")
                  for k in range(8)]
            wqk = [cpool.tile([128, 512], bf16, tag=f"wqk{k}", name=f"wqk{k}")
                   for k in range(8)]
            wv = [cpool.tile([128, 256], bf16, tag=f"wv{k}", name=f"wv{k}")
                  for k in range(8)]
            wo = [cpool.tile([128, DM], bf16, tag=f"wo{p}", name=f"wo{p}")
                  for p in range(2)]
            bqkA = cpool.tile([128, 128], bf16, tag="bqkA")
            bqkAn = cpool.tile([128, 128], bf16, tag="bqkAn")
            bqkB = cpool.tile([128, 128], bf16, tag="bqkB")
            cosr = cpool.tile([128, T], bf16, tag="cosr")
            sinr = cpool.tile([128, T], bf16, tag="sinr")
            wm4 = cpool.tile([128, 128], f32r, tag="wm4")
            i4q4 = cpool.tile([128, 128], f32r, tag="i4q4")
            i4kn2 = cpool.tile([128, 128], f32r, tag="i4kn2")
            gsum = cpool.tile([128, 128], f32r, tag="gsum")
            bsum4 = cpool.tile([128, 128], f32r, tag="bsum4")
            maskd = cpool.tile([128, 128], bf16, tag="maskd")
            ones64 = cpool.tile([1, 64], f32r, tag="ones64")
            onesv = cpool.tile([64, 2], f32r, tag="onesv")
            vext = cpool.tile([128, TC, 260], bf16, tag="vext")
            stacked = [cpool.tile([128, T], bf16, tag=f"stk{p}", name=f"stk{p}")
                       for p in range(2)]

            xt_r = xt_d.ap().rearrange("(k p) t -> k p t", p=128)
            wqk_r = wqk_d.ap().rearrange("(k p) m -> k p m", p=128)
            wv_r = wv_d.ap().rearrange("(k p) m -> k p m", p=128)
            wo_r = wo_d.ap().rearrange("(k p) m -> k p m", p=128)
            for k in range(8):
                nc.sync.dma_start(xt[k][:], xt_r[k])
                nc.sync.dma_start(wqk[k][:], wqk_r[k])
            nc.sync.dma_start(cosr[:], cosr_d.ap())
            nc.sync.dma_start(sinr[:], sinr_d.ap())
            nc.sync.dma_start(bqkA[:], bqkA_d.ap())
            nc.sync.dma_start(bqkAn[:], bqkAn_d.ap())
            nc.sync.dma_start(bqkB[:], bqkB_d.ap())
            for k in range(8):
                nc.sync.dma_start(wv[k][:], wv_r[k])
            nc.sync.dma_start(wm4[:], wm4_d.ap())
            nc.sync.dma_start(i4q4[:], i4q4_d.ap())
            nc.sync.dma_start(i4kn2[:], i4kn2_d.ap())
            nc.sync.dma_start(gsum[:], gsum_d.ap())
            nc.sync.dma_start(bsum4[:], bsum4_d.ap())
            nc.sync.dma_start(maskd[:], maskd_d.ap())
            for p in range(2):
                nc.sync.dma_start(wo[p][:], wo_r[p])
            nc.gpsimd.memset(ones64[:].bitcast(f32), 1.0)
            nc.gpsimd.memset(onesv[:].bitcast(f32), 1.0)
            nc.gpsimd.memset(vext[:], 1.0)

            def pU(name):
                return psp.tile([128, 2, 512], f32, tag="pU", bufs=2, name=name)

            def pS(shape, name):
                return psp.tile(shape, f32, tag="pS", bufs=2, name=name)

            def pO(shape, name):
                return psp.tile(shape, f32, tag="pO", bufs=2, name=name)

            # ---- QK projection: per pair one [128, 2, 512] pair tile ----
            ropes = {}
            for p in range(2):
                qk = pU(f"qk{p}")
                for s in range(2):
                    m = 2 * p + s
                    for k in range(8):
                        nc.tensor.matmul(
                            qk[:, s, :], wqk[k][:, m * 128:(m + 1) * 128],
                            xt[k][:], start=(k == 0), stop=(k == 7),
                            skip_group_check=True)
                m1 = mpool.tile([128, T], bf16, tag="m", name=f"m1_{p}")
                m2 = mpool.tile([128, T], bf16, tag="m", name=f"m2_{p}")
                m3 = mpool.tile([128, T], bf16, tag="m", name=f"m3_{p}")
                m4 = mpool.tile([128, T], bf16, tag="m", name=f"m4_{p}")
                nc.vector.tensor_mul(m1[:], qk[:, 0, :], cosr[:])
                nc.vector.tensor_mul(m2[:], qk[:, 1, :], sinr[:])
                nc.vector.tensor_mul(m3[:], qk[:, 0, :], sinr[:])
                nc.vector.tensor_mul(m4[:], qk[:, 1, :], cosr[:])
                ropes[p] = (m1, m2, m3, m4)

            # ---- V projection ----
            for jc in range(TC):
                v_ps = pO([128, 256], f"v{jc}")
                for k in range(8):
                    nc.tensor.matmul(
                        v_ps[:], xt[k][:, jc * 128:(jc + 1) * 128], wv[k][:],
                        start=(k == 0), stop=(k == 7))
                vsl = vext[:, jc, :].rearrange("p (h c) -> p h c", c=65)
                nc.vector.tensor_copy(
                    vsl[:, :, 0:64],
                    v_ps[:].rearrange("p (h c) -> p h c", c=64))

            # ---- qm/km + sigmoid ----
            sig = []
            for p in range(2):
                m1, m2, m3, m4 = ropes[p]
                qkm_ps = pS([128, T], f"qkm{p}")
                nc.tensor.matmul(qkm_ps[:], bqkA[:], m1[:], start=True, stop=False)
                nc.tensor.matmul(qkm_ps[:], bqkAn[:], m2[:], start=False, stop=False)
                nc.tensor.matmul(qkm_ps[:], bqkB[:], m3[:], start=False, stop=False)
                nc.tensor.matmul(qkm_ps[:], bqkB[:], m4[:], start=False, stop=True)
                sg = sigpool.tile([128, T], f32r, tag=f"sig{p}", name=f"sig{p}")
                nc.scalar.activation(sg[:], qkm_ps[:], Act.Sigmoid)
                sig.append(sg)

            # ---- metric stage, both pairs up front ----
            # uu = -1/2 U [(r,d), i]; km = -2 km replicated; qm replicated;
            # gt = qm + w'; ksq bias.
            pairdat = []
            for p in range(2):
                sg = sig[p]
                up = pU(f"uu{p}")
                for e in range(2):
                    nc.tensor.matmul(up[:, e, :], wm4[32 * e:32 * e + 32, :],
                                     sg[32 * e:32 * e + 32, :],
                                     start=True, stop=True,
                                     skip_group_check=True)
                uu = metpool.tile([128, 2, T], f32r, tag="uu", bufs=2,
                                  name=f"uu{p}")
                nc.vector.tensor_copy(uu[:], up[:])
                kp = pU(f"km{p}")
                for e in range(2):
                    nc.tensor.matmul(kp[:, e, :],
                                     i4kn2[64 + 32 * e:96 + 32 * e, :],
                                     sg[64 + 32 * e:96 + 32 * e, :],
                                     start=True, stop=True,
                                     tile_position=(64 + 32 * e, 0),
                                     skip_group_check=True)
                km = metpool.tile([128, 2, T], f32r, tag="km", bufs=2,
                                  name=f"km{p}")
                nc.vector.tensor_copy(km[:], kp[:])
                kmsq2 = metpool.tile([64, T], f32r, tag="kmsq", bufs=2,
                                     name=f"kmsq{p}")
                nc.scalar.activation(kmsq2[:], sg[64:128, :], Act.Square)
                biases = []
                for e in range(2):
                    ksq_ps = pS([128, 2 * TC], f"ksq{p}{e}")
                    for jc in range(TC):
                        nc.tensor.matmul(
                            ksq_ps[:, 2 * jc:2 * jc + 2],
                            kmsq2[32 * e:32 * e + 32,
                                  jc * 128:(jc + 1) * 128],
                            onesv[32 * e:32 * e + 32, 0:2],
                            start=True, stop=True, skip_group_check=True)
                    bias_h = metpool.tile([128, 2 * TC], f32, tag="bias",
                                          bufs=4, name=f"bias{p}{e}")
                    nc.scalar.mul(bias_h[:], ksq_ps[:], it)
                    biases.append(bias_h)
                gts = []
                for e in range(2):
                    qp = pS([128, T], f"qm{p}{e}")
                    nc.tensor.matmul(qp[:], i4q4[32 * e:32 * e + 32, :],
                                     sg[32 * e:32 * e + 32, :],
                                     start=True, stop=True)
                    qm = metpool.tile([128, T], f32r, tag="qm", bufs=4,
                                      name=f"qm{p}{e}")
                    nc.scalar.copy(qm[:], qp[:])
                    tmpc = metpool.tile([128, T], f32r, tag="tmpc", bufs=2,
                                        name=f"tmpc{p}{e}")
                    nc.gpsimd.tensor_mul(tmpc[:], uu[:, e, :], qm[:])
                    uq_ps = pS([128, T], f"uq{p}{e}")
                    nc.tensor.matmul(uq_ps[:], gsum[:], tmpc[:],
                                     start=True, stop=True)
                    tmp2c = metpool.tile([128, T], f32r, tag="tmp2c", bufs=2,
                                         name=f"tmp2c{p}{e}")
                    nc.vector.tensor_mul(tmp2c[:], uu[:, e, :], uq_ps[:])
                    wp_ps = pS([128, T], f"wp{p}{e}")
                    nc.tensor.matmul(wp_ps[:], bsum4[:], tmp2c[:],
                                     start=True, stop=True)
                    gt = metpool.tile([128, T], f32r, tag="gt", bufs=4,
                                      name=f"gt{p}{e}")
                    nc.vector.tensor_add(gt[:], qm[:], wp_ps[:])
                    gts.append(gt)
                pairdat.append((uu, km, gts, biases))

            # ---- scores, software-pipelined per pair ----
            for p in range(2):
                uu, km, gts, biases = pairdat[p]
                ot = [pO([128, T], f"ot{p}{e}") for e in range(2)]
                prev = None  # (e, jc, pt) blocks pending attnV
                pend = []
                for jc in range(TC):
                    ioff = 128 * jc
                    ni = T - ioff
                    blocks = []
                    for e in range(2):
                        ukA = pU(f"ukA{p}{e}{jc}")
                        for r in range(2):
                            nc.tensor.matmul(
                                ukA[:, r, :ni],
                                km[32 * r:32 * r + 32, e,
                                   ioff:ioff + 128],
                                uu[32 * r:32 * r + 32, e, ioff:],
                                start=True, stop=True,
                                skip_group_check=True)
                        sl = pS([128, T], f"sl{p}{e}{jc}")
                        nc.tensor.matmul(
                            sl[:, :ni],
                            km[64:96, e, ioff:ioff + 128],
                            gts[e][64:96, ioff:],
                            start=True, stop=True)
                        ukB = pU(f"ukB{p}{e}{jc}")
                        for r in range(2):
                            nc.tensor.matmul(
                                ukB[:, r, :ni],
                                km[64 + 32 * r:96 + 32 * r, e,
                                   ioff:ioff + 128],
                                uu[64 + 32 * r:96 + 32 * r, e, ioff:],
                                start=True, stop=True,
                                tile_position=(64 + 32 * r, 0),
                                skip_group_check=True)
                        blocks.append((e, ukA, ukB, sl))
                    # attnV for previous jc (pipelined behind the spans)
                    for (pe, ppt, pioff, pni) in pend:
                        hl = 2 * p + pe
                        nc.tensor.matmul(
                            ot[pe][:65, pioff:],
                            vext[:, pioff // 128, hl * 65:(hl + 1) * 65],
                            ppt[:, :pni],
                            start=(pioff == 0), stop=(pioff == 384),
                            skip_group_check=True)
                    pend = []
                    for (e, ukA, ukB, sl) in blocks:
                        sqA = fpool.tile([128, 2, T], bf16, tag="f",
                                         name=f"sqA{p}{e}{jc}")
                        sqB = fpool.tile([128, 2, T], bf16, tag="f",
                                         name=f"sqB{p}{e}{jc}")
                        nc.scalar.square(sqA[:, :, :ni], ukA[:, :, :ni])
                        nc.scalar.square(sqB[:, :, :ni], ukB[:, :, :ni])
                        psum = ptpool.tile([128, 2, T], bf16, tag="psum",
                                           name=f"ps{p}{e}{jc}")
                        nc.vector.tensor_add(psum[:, :, :ni], sqA[:, :, :ni],
                                             sqB[:, :, :ni])
                        tt = ptpool.tile([128, T], bf16, tag="tt",
                                         name=f"tt{p}{e}{jc}")
                        nc.gpsimd.tensor_add(tt[:, :ni], psum[:, 0, :ni],
                                             psum[:, 1, :ni])
                        nc.vector.tensor_add(sl[:, :ni], tt[:, :ni],
                                             sl[:, :ni])
                        pt = ptpool.tile([128, T], bf16, tag="pt",
                                         name=f"pt{p}{e}{jc}")
                        nc.scalar.activation(pt[:, :ni], sl[:, :ni],
                                             Act.Exp, scale=it,
                                             bias=biases[e][:, 2 * jc:2 * jc + 1])
                        nc.gpsimd.tensor_mul(pt[:, 0:128], pt[:, 0:128],
                                             maskd[:])
                        pend.append((e, pt, ioff, ni))
                # last jc attnV
                for (pe, ppt, pioff, pni) in pend:
                    hl = 2 * p + pe
                    nc.tensor.matmul(
                        ot[pe][:65, pioff:],
                        vext[:, pioff // 128, hl * 65:(hl + 1) * 65],
                        ppt[:, :pni],
                        start=(pioff == 0), stop=(pioff == 384),
                        skip_group_check=True)

                # normalize
                for e in range(2):
                    den = scpool.tile([1, T], f32r, tag="den", name=f"dn{p}{e}")
                    nc.vector.tensor_copy(den[:], ot[e][64:65, :])
                    bc_ps = pS([64, T], f"bc{p}{e}")
                    nc.tensor.matmul(bc_ps[:], ones64[:], den[:],
                                     start=True, stop=True)
                    bc = scpool.tile([64, T], f32, tag="bc", name=f"bc{p}{e}")
                    nc.vector.reciprocal_approx_fast(out=bc[:], in_=bc_ps[:])
                    nc.vector.tensor_mul(stacked[p][64 * e:64 * e + 64, :],
                                         ot[e][:64, :], bc[:])

            # ---- output projection ----
            for ic in range(TC):
                for ncn in range(2):
                    y_ps = pS([128, 512], f"y{ic}{ncn}")
                    nc.tensor.matmul(
                        y_ps[:], stacked[0][:, ic * 128:(ic + 1) * 128],
                        wo[0][:, ncn * 512:(ncn + 1) * 512],
                        start=True, stop=False)
                    nc.tensor.matmul(
                        y_ps[:], stacked[1][:, ic * 128:(ic + 1) * 128],
                        wo[1][:, ncn * 512:(ncn + 1) * 512],
                        start=False, stop=True)
                    y_sb = opool.tile([128, 512], bf16, tag="ysb",
                                      name=f"ysb{ic}{ncn}")
                    if ncn == 0:
                        nc.scalar.copy(y_sb[:], y_ps[:])
                    else:
                        nc.vector.tensor_copy(y_sb[:], y_ps[:])
                    nc.sync.dma_start(
                        y_d.ap()[ic * 128:(ic + 1) * 128,
                                 ncn * 512:(ncn + 1) * 512],
                        y_sb[:])

    nc.compile()
    return nc


def _r32(a):
    u = np.ascontiguousarray(a, np.float32).view(np.uint32).astype(np.uint64)
    u = (u + 0x7FF + ((u >> 12) & 1)) & 0xFFFFF000
    return u.astype(np.uint32).view(np.float32)


def _bf(a):
    return np.ascontiguousarray(np.asarray(a, np.float32)).astype(BF16)


def _rope_tables():
    inv_freq = 1.0 / (10000.0 ** (np.arange(0, DH, 2, dtype=np.float32) / DH))
    t = np.arange(T, dtype=np.float32)
    freqs = t[:, None] * inv_freq[None, :]
    return np.cos(freqs), np.sin(freqs)


def _prep_inputs(x, Wq, Wk, Wv, Wo, Wqm, Wkm, Wmetric, temperature):
    x = np.asarray(x, np.float32)
    Wq, Wk, Wv, Wo = (np.asarray(w, np.float32) for w in (Wq, Wk, Wv, Wo))
    Wqm, Wkm = np.asarray(Wqm, np.float32), np.asarray(Wkm, np.float32)
    Wmetric = np.asarray(Wmetric, np.float32)

    cosf, sinf = _rope_tables()
    cosr = _bf(np.tile(cosf.T, (4, 1)))
    sinr = _bf(np.tile(sinf.T, (4, 1)))

    bqkA = np.zeros((128, 128), np.float32)
    bqkB = np.zeros((128, 128), np.float32)
    for ee in range(2):
        bqkA[64 * ee:64 * ee + 32, 32 * ee:32 * ee + 32] = Wqm[0:32]
        bqkA[64 * ee + 32:64 * ee + 64, 64 + 32 * ee:96 + 32 * ee] = Wkm[0:32]
        bqkB[64 * ee:64 * ee + 32, 32 * ee:32 * ee + 32] = Wqm[32:64]
        bqkB[64 * ee + 32:64 * ee + 64, 64 + 32 * ee:96 + 32 * ee] = Wkm[32:64]

    wm = -0.5 * np.ascontiguousarray(
        Wmetric.reshape(D, D, R).transpose(0, 2, 1).reshape(D, D * R))
    wm4 = _r32(np.tile(wm, (4, 1)))

    i4 = np.tile(np.eye(D, dtype=np.float32), (1, 4))
    i4q4 = _r32(np.tile(i4, (4, 1)))
    i4kn2 = _r32(np.tile(-2.0 * i4, (4, 1)))

    gsum = np.zeros((128, 128), np.float32)
    for a in range(128):
        for bcol in range(128):
            if a // 32 == bcol // 32:
                gsum[a, bcol] = 1.0
    bsum4 = np.zeros((128, 128), np.float32)
    for a in range(128):
        for m in range(128):
            if a % 32 == m % 32:
                bsum4[a, m] = 4.0

    jj, ii = np.meshgrid(np.arange(128), np.arange(128), indexing="ij")
    maskd = _bf((jj <= ii).astype(np.float32))

    in_maps = []
    for c in range(NCORE):
        b, g = c // 4, c % 4
        lh0 = 4 * g
        wqk = np.empty((DM, 512), np.float32)
        for p in range(2):
            for s in range(2):
                m = 2 * p + s
                for ee in range(2):
                    h = lh0 + 2 * p + ee
                    cq = Wq[:, h * 64 + 32 * s: h * 64 + 32 * s + 32]
                    ck = Wk[:, h * 64 + 32 * s: h * 64 + 32 * s + 32]
                    wqk[:, m * 128 + 64 * ee: m * 128 + 64 * ee + 32] = cq
                    wqk[:, m * 128 + 64 * ee + 32: m * 128 + 64 * ee + 64] = ck
        in_maps.append({
            "xt": _bf(x[b].T),
            "wqk": _bf(wqk),
            "wv": _bf(Wv[:, lh0 * 64: lh0 * 64 + 256]),
            "wo": _bf(Wo[lh0 * 64: lh0 * 64 + 256, :]),
            "bqkA": _bf(bqkA),
            "bqkAn": _bf(-bqkA),
            "bqkB": _bf(bqkB),
            "cosr": cosr,
            "sinr": sinr,
            "wm4": wm4,
            "i4q4": i4q4,
            "i4kn2": i4kn2,
            "gsum": gsum,
            "bsum4": bsum4,
            "maskd": maskd,
        })
    return in_maps


def kernel(x, Wq, Wk, Wv, Wo, Wqm, Wkm, Wmetric, temperature, **_):
    from concourse import bass_utils

    temp = float(np.asarray(temperature))
    key = ("nc", temp)
    if key not in _CACHE:
        _CACHE[key] = _build(temp)
        _CACHE["nc"] = _CACHE[key]
    nc = _CACHE[key]

    in_maps = _prep_inputs(x, Wq, Wk, Wv, Wo, Wqm, Wkm, Wmetric, temperature)
    res = bass_utils.run_bass_kernel_spmd(nc, in_maps,
                                          core_ids=list(range(NCORE)))
    y = np.zeros((B, T, DM), np.float32)
    for b in range(B):
        acc = res.results[4 * b]["y"].astype(np.float32)
        for g in range(1, 4):
            acc = acc + res.results[4 * b + g]["y"].astype(np.float32)
        y[b] = acc
    return y


# revision 3
# speedup vs baseline: 1.2398x; 1.0383x over previous
"""DRM attention kernel for 8 Trainium2 NeuronCores — v10.

v2 -> v3: pair-tile PSUM layout (2-bank tiles) so ACT squares cover two
uk banks per instruction; all-pairs metric stage hoisted before scores;
software-pipelined score loop (spans -> squares -> tree -> exp -> attnV
one block behind) to keep the PE stream dense for HAM; copies spread
across DVE/ACT/GPS by phase.
"""

import numpy as np
import ml_dtypes

B, T, DM = 2, 512, 1024
H, DH = 16, 64
D, R = 32, 4
TEMP_MIN = 0.5
NCORE = 8
TC = 4

_CACHE = {}
BF16 = ml_dtypes.bfloat16


def _build(temp):
    import concourse.bass as bass
    import concourse.tile as tile
    from concourse import mybir, bacc

    f32 = mybir.dt.float32
    f32r = mybir.dt.float32r
    bf16 = mybir.dt.bfloat16
    PSUM = bass.MemorySpace.PSUM
    Act = mybir.ActivationFunctionType

    it = -1.0 / max(temp, TEMP_MIN)

    nc = bacc.Bacc("TRN2", target_bir_lowering=False, debug=False)

    xt_d = nc.dram_tensor("xt", [DM, T], bf16, kind="ExternalInput")
    wqk_d = nc.dram_tensor("wqk", [DM, 512], bf16, kind="ExternalInput")
    wv_d = nc.dram_tensor("wv", [DM, 256], bf16, kind="ExternalInput")
    wo_d = nc.dram_tensor("wo", [256, DM], bf16, kind="ExternalInput")
    bqkA_d = nc.dram_tensor("bqkA", [128, 128], bf16, kind="ExternalInput")
    bqkAn_d = nc.dram_tensor("bqkAn", [128, 128], bf16, kind="ExternalInput")
    bqkB_d = nc.dram_tensor("bqkB", [128, 128], bf16, kind="ExternalInput")
    cosr_d = nc.dram_tensor("cosr", [128, T], bf16, kind="ExternalInput")
    sinr_d = nc.dram_tensor("sinr", [128, T], bf16, kind="ExternalInput")
    wm4_d = nc.dram_tensor("wm4", [128, 128], f32r, kind="ExternalInput")
    i4q4_d = nc.dram_tensor("i4q4", [128, 128], f32r, kind="ExternalInput")
    i4kn2_d = nc.dram_tensor("i4kn2", [128, 128], f32r, kind="ExternalInput")
    gsum_d = nc.dram_tensor("gsum", [128, 128], f32r, kind="ExternalInput")
    bsum4_d = nc.dram_tensor("bsum4", [128, 128], f32r, kind="ExternalInput")
    maskd_d = nc.dram_tensor("maskd", [128, 128], bf16, kind="ExternalInput")
    y_d = nc.dram_tensor("y", [T, DM], bf16, kind="ExternalOutput")

    with tile.TileContext(nc) as tc:
        with (
            tc.tile_pool(name="const", bufs=1) as cpool,
            tc.tile_pool(name="sig", bufs=1) as sigpool,
            tc.tile_pool(name="m", bufs=8) as mpool,
            tc.tile_pool(name="met", bufs=2) as metpool,
            tc.tile_pool(name="sc", bufs=2) as scpool,
            tc.tile_pool(name="f", bufs=4) as fpool,
            tc.tile_pool(name="pt", bufs=4) as ptpool,
            tc.tile_pool(name="out", bufs=2) as opool,
            tc.tile_pool(name="ps", bufs=1, space=PSUM) as psp,
        ):
            # ---- constants / weights ----
            xt = [cpool.tile([128, T], bf16, tag=f"xt{k}", name=f"xt{k}")
                  for k in range(8)]
            wqk = [cpool.tile([128, 512], bf16, tag=f"wqk{k}", name=f"wqk{k}")
                   for k in range(8)]
            wv = [cpool.tile([128, 256], bf16, tag=f"wv{k}", name=f"wv{k}")
                  for k in range(8)]
            wo = [cpool.tile([128, DM], bf16, tag=f"wo{p}", name=f"wo{p}")
                  for p in range(2)]
            bqkA = cpool.tile([128, 128], bf16, tag="bqkA")
            bqkAn = cpool.tile([128, 128], bf16, tag="bqkAn")
            bqkB = cpool.tile([128, 128], bf16, tag="bqkB")
            cosr = cpool.tile([128, T], bf16, tag="cosr")
            sinr = cpool.tile([128, T], bf16, tag="sinr")
            wm4 = cpool.tile([128, 128], f32r, tag="wm4")
            i4q4 = cpool.tile([128, 128], f32r, tag="i4q4")
            i4kn2 = cpool.tile([128, 128], f32r, tag="i4kn2")
            gsum = cpool.tile([128, 128], f32r, tag="gsum")
            bsum4 = cpool.tile([128, 128], f32r, tag="bsum4")
            maskd = cpool.tile([128, 128], bf16, tag="maskd")
            ones64 = cpool.tile([1, 64], f32r, tag="ones64")
            onesv = cpool.tile([64, 2], f32r, tag="onesv")
            vext = cpool.tile([128, TC, 260], bf16, tag="vext")
            stacked = [cpool.tile([128, T], bf16, tag=f"stk{p}", name=f"stk{p}")
                       for p in range(2)]

            xt_r = xt_d.ap().rearrange("(k p) t -> k p t", p=128)
            wqk_r = wqk_d.ap().rearrange("(k p) m -> k p m", p=128)
            wv_r = wv_d.ap().rearrange("(k p) m -> k p m", p=128)
            wo_r = wo_d.ap().rearrange("(k p) m -> k p m", p=128)
            for k in range(8):
                nc.sync.dma_start(xt[k][:], xt_r[k])
                nc.sync.dma_start(wqk[k][:], wqk_r[k])
            nc.sync.dma_start(cosr[:], cosr_d.ap())
            nc.sync.dma_start(sinr[:], sinr_d.ap())
            nc.sync.dma_start(bqkA[:], bqkA_d.ap())
            nc.sync.dma_start(bqkAn[:], bqkAn_d.ap())
            nc.sync.dma_start(bqkB[:], bqkB_d.ap())
            for k in range(8):
                nc.sync.dma_start(wv[k][:], wv_r[k])
            nc.sync.dma_start(wm4[:], wm4_d.ap())
            nc.sync.dma_start(i4q4[:], i4q4_d.ap())
            nc.sync.dma_start(i4kn2[:], i4kn2_d.ap())
            nc.sync.dma_start(gsum[:], gsum_d.ap())
            nc.sync.dma_start(bsum4[:], bsum4_d.ap())
            nc.sync.dma_start(maskd[:], maskd_d.ap())
            for p in range(2):
                nc.sync.dma_start(wo[p][:], wo_r[p])
            nc.gpsimd.memset(ones64[:].bitcast(f32), 1.0)
            nc.gpsimd.memset(onesv[:].bitcast(f32), 1.0)
            nc.gpsimd.memset(vext[:], 1.0)

            def pU(name):
                return psp.tile([128, 2, 512], f32, tag="pU", bufs=2, name=name)

            def pS(shape, name):
                return psp.tile(shape, f32, tag="pS", bufs=2, name=name)

            def pO(shape, name):
                return psp.tile(shape, f32, tag="pO", bufs=2, name=name)

            # ---- QK projection: per pair one [128, 2, 512] pair tile ----
            ropes = {}
            for p in range(2):
                qk = pU(f"qk{p}")
                for s in range(2):
                    m = 2 * p + s
                    for k in range(8):
                        nc.tensor.matmul(
                            qk[:, s, :], wqk[k][:, m * 128:(m + 1) * 128],
                            xt[k][:], start=(k == 0), stop=(k == 7),
                            skip_group_check=True)
                m1 = mpool.tile([128, T], bf16, tag="m", name=f"m1_{p}")
                m2 = mpool.tile([128, T], bf16, tag="m", name=f"m2_{p}")
                m3 = mpool.tile([128, T], bf16, tag="m", name=f"m3_{p}")
                m4 = mpool.tile([128, T], bf16, tag="m", name=f"m4_{p}")
                nc.vector.tensor_mul(m1[:], qk[:, 0, :], cosr[:])
                nc.vector.tensor_mul(m2[:], qk[:, 1, :], sinr[:])
                nc.vector.tensor_mul(m3[:], qk[:, 0, :], sinr[:])
                nc.vector.tensor_mul(m4[:], qk[:, 1, :], cosr[:])
                ropes[p] = (m1, m2, m3, m4)

            # ---- V projection ----
            for jc in range(TC):
                v_ps = pO([128, 256], f"v{jc}")
                for k in range(8):
                    nc.tensor.matmul(
                        v_ps[:], xt[k][:, jc * 128:(jc + 1) * 128], wv[k][:],
                        start=(k == 0), stop=(k == 7))
                vsl = vext[:, jc, :].rearrange("p (h c) -> p h c", c=65)
                nc.vector.tensor_copy(
                    vsl[:, :, 0:64],
                    v_ps[:].rearrange("p (h c) -> p h c", c=64))

            # ---- qm/km + sigmoid ----
            sig = []
            for p in range(2):
                m1, m2, m3, m4 = ropes[p]
                qkm_ps = pS([128, T], f"qkm{p}")
                nc.tensor.matmul(qkm_ps[:], bqkA[:], m1[:], start=True, stop=False)
                nc.tensor.matmul(qkm_ps[:], bqkAn[:], m2[:], start=False, stop=False)
                nc.tensor.matmul(qkm_ps[:], bqkB[:], m3[:], start=False, stop=False)
                nc.tensor.matmul(qkm_ps[:], bqkB[:], m4[:], start=False, stop=True)
                sg = sigpool.tile([128, T], f32r, tag=f"sig{p}", name=f"sig{p}")
                nc.scalar.activation(sg[:], qkm_ps[:], Act.Sigmoid)
                sig.append(sg)

            # ---- metric stage, both pairs up front ----
            # uu = -1/2 U [(r,d), i]; km = -2 km replicated; qm replicated;
            # gt = qm + w'; ksq bias.
            pairdat = []
            for p in range(2):
                sg = sig[p]
                up = pU(f"uu{p}")
                for e in range(2):
                    nc.tensor.matmul(up[:, e, :], wm4[32 * e:32 * e + 32, :],
                                     sg[32 * e:32 * e + 32, :],
                                     start=True, stop=True,
                                     skip_group_check=True)
                uu = metpool.tile([128, 2, T], f32r, tag="uu", bufs=2,
                                  name=f"uu{p}")
                nc.vector.tensor_copy(uu[:], up[:])
                kp = pU(f"km{p}")
                for e in range(2):
                    nc.tensor.matmul(kp[:, e, :],
                                     i4kn2[64 + 32 * e:96 + 32 * e, :],
                                     sg[64 + 32 * e:96 + 32 * e, :],
                                     start=True, stop=True,
                                     tile_position=(64 + 32 * e, 0),
                                     skip_group_check=True)
                km = metpool.tile([128, 2, T], f32r, tag="km", bufs=2,
                                  name=f"km{p}")
                nc.vector.tensor_copy(km[:], kp[:])
                kmsq2 = metpool.tile([64, T], f32r, tag="kmsq", bufs=2,
                                     name=f"kmsq{p}")
                nc.scalar.activation(kmsq2[:], sg[64:128, :], Act.Square)
                biases = []
                for e in range(2):
                    ksq_ps = pS([128, 2 * TC], f"ksq{p}{e}")
                    for jc in range(TC):
                        nc.tensor.matmul(
                            ksq_ps[:, 2 * jc:2 * jc + 2],
                            kmsq2[32 * e:32 * e + 32,
                                  jc * 128:(jc + 1) * 128],
                            onesv[32 * e:32 * e + 32, 0:2],
                            start=True, stop=True, skip_group_check=True)
                    bias_h = metpool.tile([128, 2 * TC], f32, tag="bias",
                                          bufs=4, name=f"bias{p}{e}")
                    nc.scalar.mul(bias_h[:], ksq_ps[:], it)
                    biases.append(bias_h)
                gts = []
                for e in range(2):
                    qp = pS([128, T], f"qm{p}{e}")
                    nc.tensor.matmul(qp[:], i4q4[32 * e:32 * e + 32, :],
                                     sg[32 * e:32 * e + 32, :],
                                     start=True, stop=True)
                    qm = metpool.tile([128, T], f32r, tag="qm", bufs=4,
                                      name=f"qm{p}{e}")
                    nc.scalar.copy(qm[:], qp[:])
                    tmpc = metpool.tile([128, T], f32r, tag="tmpc", bufs=2,
                                        name=f"tmpc{p}{e}")
                    nc.gpsimd.tensor_mul(tmpc[:], uu[:, e, :], qm[:])
                    uq_ps = pS([128, T], f"uq{p}{e}")
                    nc.tensor.matmul(uq_ps[:], gsum[:], tmpc[:],
                                     start=True, stop=True)
                    tmp2c = metpool.tile([128, T], f32r, tag="tmp2c", bufs=2,
                                         name=f"tmp2c{p}{e}")
                    nc.vector.tensor_mul(tmp2c[:], uu[:, e, :], uq_ps[:])
                    wp_ps = pS([128, T], f"wp{p}{e}")
                    nc.tensor.matmul(wp_ps[:], bsum4[:], tmp2c[:],
                                     start=True, stop=True)
                    gt = metpool.tile([128, T], f32r, tag="gt", bufs=4,
                                      name=f"gt{p}{e}")
                    nc.vector.tensor_add(gt[:], qm[:], wp_ps[:])
                    gts.append(gt)
                pairdat.append((uu, km, gts, biases))

            # ---- scores: exp and attnV software-pipelined one block
            # behind the spans/squares/tree so the ACT FIFO never stalls
            # head-of-line on a tree dependency ----
            for p in range(2):
                uu, km, gts, biases = pairdat[p]
                ot = {}
                for e in range(2):
                    ot[e] = pO([128, T], f"ot{p}{e}", )
                expq = []   # blocks awaiting exp
                attq = []   # blocks awaiting attnV
                def flush_att():
                    while attq:
                        (ae, apt, aioff, ani) = attq.pop(0)
                        hl = 2 * p + ae
                        nc.tensor.matmul(
                            ot[ae][:65, aioff:],
                            vext[:, aioff // 128, hl * 65:(hl + 1) * 65],
                            apt[:, :ani],
                            start=(aioff == 0), stop=(aioff == 384),
                            skip_group_check=True)
                def flush_exp():
                    (xe, xsl, xioff, xni, xjc) = expq.pop(0)
                    pt = ptpool.tile([128, T], bf16, tag="pt",
                                     name=f"pt{p}{xe}{xjc}")
                    nc.scalar.activation(pt[:, :xni], xsl[:, :xni],
                                         Act.Exp, scale=it,
                                         bias=biases[xe][:, 2 * xjc:2 * xjc + 1])
                    nc.gpsimd.tensor_mul(pt[:, 0:128], pt[:, 0:128],
                                         maskd[:])
                    attq.append((xe, pt, xioff, xni))
                for jc in range(TC):
                    ioff = 128 * jc
                    ni = T - ioff
                    for e in range(2):
                        ukA = pU(f"ukA{p}{e}{jc}")
                        for r in range(2):
                            nc.tensor.matmul(
                                ukA[:, r, :ni],
                                km[32 * r:32 * r + 32, e,
                                   ioff:ioff + 128],
                                uu[32 * r:32 * r + 32, e, ioff:],
                                start=True, stop=True,
                                skip_group_check=True)
                        sl = pS([128, T], f"sl{p}{e}{jc}")
                        nc.tensor.matmul(
                            sl[:, :ni],
                            km[64:96, e, ioff:ioff + 128],
                            gts[e][64:96, ioff:],
                            start=True, stop=True)
                        ukB = pU(f"ukB{p}{e}{jc}")
                        for r in range(2):
                            nc.tensor.matmul(
                                ukB[:, r, :ni],
                                km[64 + 32 * r:96 + 32 * r, e,
                                   ioff:ioff + 128],
                                uu[64 + 32 * r:96 + 32 * r, e, ioff:],
                                start=True, stop=True,
                                tile_position=(64 + 32 * r, 0),
                                skip_group_check=True)
                        # attnV from two blocks ago fills the PE here
                        flush_att()
                        sqA = fpool.tile([128, 2, T], bf16, tag="f",
                                         name=f"sqA{p}{e}{jc}")
                        sqB = fpool.tile([128, 2, T], bf16, tag="f",
                                         name=f"sqB{p}{e}{jc}")
                        nc.scalar.square(sqA[:, :, :ni], ukA[:, :, :ni])
                        nc.scalar.square(sqB[:, :, :ni], ukB[:, :, :ni])
                        psum = ptpool.tile([128, 2, T], bf16, tag="psum",
                                           name=f"ps{p}{e}{jc}")
                        nc.vector.tensor_add(psum[:, :, :ni], sqA[:, :, :ni],
                                             sqB[:, :, :ni])
                        tt = ptpool.tile([128, T], bf16, tag="tt",
                                         name=f"tt{p}{e}{jc}")
                        nc.gpsimd.tensor_add(tt[:, :ni], psum[:, 0, :ni],
                                             psum[:, 1, :ni])
                        nc.vector.tensor_add(sl[:, :ni], tt[:, :ni],
                                             sl[:, :ni])
                        expq.append((e, sl, ioff, ni, jc))
                        # exp for the PREVIOUS block (ACT stays fed with
                        # this block's squares while its tree completes)
                        if len(expq) > 1:
                            flush_exp()
                while expq:
                    flush_exp()
                flush_att()

                # normalize
                for e in range(2):
                    den = scpool.tile([1, T], f32r, tag="den", name=f"dn{p}{e}")
                    nc.vector.tensor_copy(den[:], ot[e][64:65, :])
                    bc_ps = pS([64, T], f"bc{p}{e}")
                    nc.tensor.matmul(bc_ps[:], ones64[:], den[:],
                                     start=True, stop=True)
                    bc = scpool.tile([64, T], f32, tag="bc", name=f"bc{p}{e}")
                    nc.vector.reciprocal_approx_fast(out=bc[:], in_=bc_ps[:])
                    nc.vector.tensor_mul(stacked[p][64 * e:64 * e + 64, :],
                                         ot[e][:64, :], bc[:])

            # ---- output projection ----
            for ic in range(TC):
                for ncn in range(2):
                    y_ps = pS([128, 512], f"y{ic}{ncn}")
                    nc.tensor.matmul(
                        y_ps[:], stacked[0][:, ic * 128:(ic + 1) * 128],
                        wo[0][:, ncn * 512:(ncn + 1) * 512],
                        start=True, stop=False)
                    nc.tensor.matmul(
                        y_ps[:], stacked[1][:, ic * 128:(ic + 1) * 128],
                        wo[1][:, ncn * 512:(ncn + 1) * 512],
                        start=False, stop=True)
                    y_sb = opool.tile([128, 512], bf16, tag="ysb",
                                      name=f"ysb{ic}{ncn}")
                    if ncn == 0:
                        nc.scalar.copy(y_sb[:], y_ps[:])
                    else:
                        nc.vector.tensor_copy(y_sb[:], y_ps[:])
                    nc.sync.dma_start(
                        y_d.ap()[ic * 128:(ic + 1) * 128,
                                 ncn * 512:(ncn + 1) * 512],
                        y_sb[:])

    nc.compile()
    return nc


def _r32(a):
    u = np.ascontiguousarray(a, np.float32).view(np.uint32).astype(np.uint64)
    u = (u + 0x7FF + ((u >> 12) & 1)) & 0xFFFFF000
    return u.astype(np.uint32).view(np.float32)


def _bf(a):
    return np.ascontiguousarray(np.asarray(a, np.float32)).astype(BF16)


def _rope_tables():
    inv_freq = 1.0 / (10000.0 ** (np.arange(0, DH, 2, dtype=np.float32) / DH))
    t = np.arange(T, dtype=np.float32)
    freqs = t[:, None] * inv_freq[None, :]
    return np.cos(freqs), np.sin(freqs)


def _prep_inputs(x, Wq, Wk, Wv, Wo, Wqm, Wkm, Wmetric, temperature):
    x = np.asarray(x, np.float32)
    Wq, Wk, Wv, Wo = (np.asarray(w, np.float32) for w in (Wq, Wk, Wv, Wo))
    Wqm, Wkm = np.asarray(Wqm, np.float32), np.asarray(Wkm, np.float32)
    Wmetric = np.asarray(Wmetric, np.float32)

    cosf, sinf = _rope_tables()
    cosr = _bf(np.tile(cosf.T, (4, 1)))
    sinr = _bf(np.tile(sinf.T, (4, 1)))

    bqkA = np.zeros((128, 128), np.float32)
    bqkB = np.zeros((128, 128), np.float32)
    for ee in range(2):
        bqkA[64 * ee:64 * ee + 32, 32 * ee:32 * ee + 32] = Wqm[0:32]
        bqkA[64 * ee + 32:64 * ee + 64, 64 + 32 * ee:96 + 32 * ee] = Wkm[0:32]
        bqkB[64 * ee:64 * ee + 32, 32 * ee:32 * ee + 32] = Wqm[32:64]
        bqkB[64 * ee + 32:64 * ee + 64, 64 + 32 * ee:96 + 32 * ee] = Wkm[32:64]

    wm = -0.5 * np.ascontiguousarray(
        Wmetric.reshape(D, D, R).transpose(0, 2, 1).reshape(D, D * R))
    wm4 = _r32(np.tile(wm, (4, 1)))

    i4 = np.tile(np.eye(D, dtype=np.float32), (1, 4))
    i4q4 = _r32(np.tile(i4, (4, 1)))
    i4kn2 = _r32(np.tile(-2.0 * i4, (4, 1)))

    gsum = np.zeros((128, 128), np.float32)
    for a in range(128):
        for bcol in range(128):
            if a // 32 == bcol // 32:
                gsum[a, bcol] = 1.0
    bsum4 = np.zeros((128, 128), np.float32)
    for a in range(128):
        for m in range(128):
            if a % 32 == m % 32:
                bsum4[a, m] = 4.0

    jj, ii = np.meshgrid(np.arange(128), np.arange(128), indexing="ij")
    maskd = _bf((jj <= ii).astype(np.float32))

    in_maps = []
    for c in range(NCORE):
        b, g = c // 4, c % 4
        lh0 = 4 * g
        wqk = np.empty((DM, 512), np.float32)
        for p in range(2):
            for s in range(2):
                m = 2 * p + s
                for ee in range(2):
                    h = lh0 + 2 * p + ee
                    cq = Wq[:, h * 64 + 32 * s: h * 64 + 32 * s + 32]
                    ck = Wk[:, h * 64 + 32 * s: h * 64 + 32 * s + 32]
                    wqk[:, m * 128 + 64 * ee: m * 128 + 64 * ee + 32] = cq
                    wqk[:, m * 128 + 64 * ee + 32: m * 128 + 64 * ee + 64] = ck
        in_maps.append({
            "xt": _bf(x[b].T),
            "wqk": _bf(wqk),
            "wv": _bf(Wv[:, lh0 * 64: lh0 * 64 + 256]),
            "wo": _bf(Wo[lh0 * 64: lh0 * 64 + 256, :]),
            "bqkA": _bf(bqkA),
            "bqkAn": _bf(-bqkA),
            "bqkB": _bf(bqkB),
            "cosr": cosr,
            "sinr": sinr,
            "wm4": wm4,
            "i4q4": i4q4,
            "i4kn2": i4kn2,
            "gsum": gsum,
            "bsum4": bsum4,
            "maskd": maskd,
        })
    return in_maps


def kernel(x, Wq, Wk, Wv, Wo, Wqm, Wkm, Wmetric, temperature, **_):
    from concourse import bass_utils

    temp = float(np.asarray(temperature))
    key = ("nc", temp)
    if key not in _CACHE:
        _CACHE[key] = _build(temp)
        _CACHE["nc"] = _CACHE[key]
    nc = _CACHE[key]

    in_maps = _prep_inputs(x, Wq, Wk, Wv, Wo, Wqm, Wkm, Wmetric, temperature)
    res = bass_utils.run_bass_kernel_spmd(nc, in_maps,
                                          core_ids=list(range(NCORE)))
    y = np.zeros((B, T, DM), np.float32)
    for b in range(B):
        acc = res.results[4 * b]["y"].astype(np.float32)
        for g in range(1, 4):
            acc = acc + res.results[4 * b + g]["y"].astype(np.float32)
        y[b] = acc
    return y
